# revision 1
# baseline (speedup 1.0000x reference)
"""Trainium2 Bass kernel for nn_Disc_edge2 (3-layer dense-graph GNN + MLP head).

Sharding: data-parallel over batch B=16 across 8 cores (2 graphs/core).
Per-graph on-chip layout: msg tensors are [d=128 partitions, f=16384 free]
with free index f = c1*2048 + t*128 + p  where the edge (i, j) maps to
p = i (inner 128) and j = 8*t + c1 (c1 = j%8, t = j//8).

This layout makes both per-node broadcast adds pure matmuls with CONSTANT
moving operands:
  - xi[i,:] broadcast over j  -> rhs = tiled identity [I I I I] (i = p inner)
  - xj[j,:] broadcast over i  -> rhs = SELJM[j', f] = (j' == 8t+c1), built
    once with one affine_select.
The adjacency mask is a replicated tensor built with a 0-step broadcast DMA
and applied with one bf16 tensor_tensor per chunk (scalar_tensor_tensor with
accum_out on layer 2, where the per-chunk sums become the readout mean).
The residual blends are plain adds; their 0.5 factors are folded into the
layer-2 weights.
"""

import os
import sys

sys.path.insert(0, "/opt/trn_rl_repo")

import numpy as np

import concourse.bass as bass
from concourse import bacc
import concourse.mybir as mybir
import concourse.tile as tile
from concourse.masks import make_identity

F32 = mybir.dt.float32
BF16 = mybir.dt.bfloat16
I32 = mybir.dt.int32
AF = mybir.ActivationFunctionType
OP = mybir.AluOpType

B, N, DN0, DE0, DH = 16, 128, 64, 16, 128
NCORES = 8
GPC = B // NCORES          # graphs per core
FREE = N * N               # 16384
CH = 512                   # columns per PSUM chunk
NCH = FREE // CH           # 32 chunks

WEIGHT_NAMES = [
    "w_msg_0", "b_msg_0", "w_node_0", "b_node_0",
    "w_msg_1", "b_msg_1", "w_node_1", "b_node_1",
    "w_msg_2", "b_msg_2", "w_node_2", "b_node_2",
    "w_h1", "b_h1", "w_h2", "b_h2", "w_h3", "b_h3",
]

_CACHE = {}


def build_nc():
    nc = bacc.Bacc()

    ei_d = nc.declare_dram_parameter("edge_index", [GPC, N, N], I32, isOutput=False)
    x_d = nc.declare_dram_parameter("x", [GPC, N, DN0], F32, isOutput=False)
    ea_d = nc.declare_dram_parameter("edge_attr", [GPC, N, N, DE0], F32, isOutput=False)
    wd = {}
    shapes = {
        "w_msg_0": [2 * DN0 + DE0, DH], "b_msg_0": [DH],
        "w_node_0": [DN0 + DH, DH], "b_node_0": [DH],
        "w_msg_1": [3 * DH, DH], "b_msg_1": [DH],
        "w_node_1": [2 * DH, DH], "b_node_1": [DH],
        "w_msg_2": [3 * DH, DH], "b_msg_2": [DH],
        "w_node_2": [2 * DH, DH], "b_node_2": [DH],
        "w_h1": [DH, DH], "b_h1": [DH],
        "w_h2": [DH, DH], "b_h2": [DH],
        "w_h3": [DH, 1], "b_h3": [1],
    }
    for n_ in WEIGHT_NAMES:
        wd[n_] = nc.declare_dram_parameter(n_, shapes[n_], F32, isOutput=False)
    out_d = nc.declare_dram_parameter("out", [GPC, 1], F32, isOutput=True)

    with tile.TileContext(nc) as tc:
        import contextlib
        stack = contextlib.ExitStack()
        consts = stack.enter_context(tc.tile_pool(name="consts", bufs=1))
        gbuf = stack.enter_context(tc.tile_pool(name="gbuf", bufs=1))
        small = stack.enter_context(tc.tile_pool(name="small", bufs=2))
        zpool = stack.enter_context(tc.tile_pool(name="zp", bufs=5, space="PSUM"))
        spsum = stack.enter_context(tc.tile_pool(name="sp", bufs=2, space="PSUM"))
        dpool = stack.enter_context(tc.tile_pool(name="dp", bufs=1, space="DRAM"))

        # -------- input loads first: head of the sync HWDGE FIFO --------
        e0nat, x0in, aiin = [], [], []
        for g in range(GPC):
            t = gbuf.tile([128, 128], I32, tag="ai")
            nc.sync.dma_start(t[:], ei_d[g])
            aiin.append(t)
            t = gbuf.tile([128, DN0], F32, tag=f"x0_{g}")
            nc.sync.dma_start(t[:], x_d[g])
            x0in.append(t)
        for g in range(GPC):
            t = gbuf.tile([128, N * DE0], F32, tag="e0nat")
            nc.sync.dma_start(t[:], ea_d[g].rearrange("i j k -> i (j k)"))
            e0nat.append(t)

        # -------- constants / weights (scalar-queue DMAs) --------
        def f2b(src_ap, p, name, scale=None):
            tmp = consts.tile([p, 128], F32, tag=f"tmp_{name}")
            nc.scalar.dma_start(tmp[:], src_ap)
            t = consts.tile([p, 128], BF16, tag=name)
            if scale is None:
                nc.vector.tensor_copy(t[:], tmp[:])
            else:
                nc.vector.tensor_scalar_mul(t[:], tmp[:], scale)
            return t

        ident = consts.tile([128, 128], F32, tag="ident")
        make_identity(nc, ident[:])

        w = {}
        w["Wi0"] = f2b(wd["w_msg_0"][0:DN0, :], DN0, "Wi0")
        w["Wj0"] = f2b(wd["w_msg_0"][DN0:2 * DN0, :], DN0, "Wj0")
        # 8 block variants of We0: rows [c1*16, c1*16+16) = We0, rest zero, so
        # the K=128 contraction against E0T[(cc,k), (t,p)] picks out cc == c1.
        we0b16 = f2b(wd["w_msg_0"][2 * DN0:, :], DE0, "we0b16")
        we0blk = []
        for c1 in range(8):
            blk = consts.tile([128, DH], BF16, tag=f"we0b{c1}")
            nc.vector.memset(blk[:], 0.0)
            nc.scalar.dma_start(blk[c1 * 16:(c1 + 1) * 16, :], we0b16[:])
            we0blk.append(blk)

        # tiled identity [I I I I] bf16: seli[p', (u,p)] = (p'==p)
        seli = consts.tile([128, CH], BF16, tag="seli")
        nc.gpsimd.memset(seli[:], 0.0)
        nc.gpsimd.affine_select(
            out=seli[:], in_=seli[:], compare_op=OP.not_equal, fill=1.0,
            base=0, pattern=[[0, 4], [-1, 128]], channel_multiplier=1)

        # SELJM[j', (c1,t,p)] = (j' == 8t + c1)
        seljm = consts.tile([128, FREE], BF16, tag="seljm")
        nc.gpsimd.memset(seljm[:], 0.0)
        nc.gpsimd.affine_select(
            out=seljm[:], in_=seljm[:], compare_op=OP.not_equal, fill=1.0,
            base=0, pattern=[[-1, 8], [-8, 16], [0, 128]], channel_multiplier=1)

        w["Wx0"] = f2b(wd["w_node_0"][0:DN0, :], DN0, "Wx0")
        w["Wa0"] = f2b(wd["w_node_0"][DN0:, :], DH, "Wa0")
        for l in (1, 2):
            sc = 0.5 if l == 2 else None
            w[f"Wi{l}"] = f2b(wd[f"w_msg_{l}"][0:DH, :], DH, f"Wi{l}", sc)
            w[f"Wj{l}"] = f2b(wd[f"w_msg_{l}"][DH:2 * DH, :], DH, f"Wj{l}", sc)
            w[f"We{l}"] = f2b(wd[f"w_msg_{l}"][2 * DH:, :], DH, f"We{l}", sc)
        w["Wx1"] = f2b(wd["w_node_1"][0:DH, :], DH, "Wx1")
        w["Wa1"] = f2b(wd["w_node_1"][DH:, :], DH, "Wa1")
        # head weights stay f32
        wh1 = consts.tile([DH, DH], F32, tag="wh1")
        nc.scalar.dma_start(wh1[:], wd["w_h1"][:, :])
        wh2 = consts.tile([DH, DH], F32, tag="wh2")
        nc.scalar.dma_start(wh2[:], wd["w_h2"][:, :])
        wh3 = consts.tile([DH, 1], F32, tag="wh3")
        nc.scalar.dma_start(wh3[:], wd["w_h3"][:, :])

        # bias rows replicated across partitions via 0-step broadcast DMA
        brep = {}
        for l in range(3):
            rep = consts.tile([128, DH], F32, tag=f"brep{l}")
            nc.scalar.dma_start(
                rep[:], wd[f"b_msg_{l}"][:].unsqueeze(0).to_broadcast([128, DH]))
            brep[l] = rep
        bcol = {}
        for nm in ("b_node_0", "b_node_1", "b_h1", "b_h2"):
            c = consts.tile([DH, 1], F32, tag=f"col_{nm}")
            nc.scalar.dma_start(c[:], wd[nm][:].unsqueeze(1))
            bcol[nm] = c
        bh3 = consts.tile([1, 1], F32, tag="col_bh3")
        nc.scalar.dma_start(bh3[:], wd["b_h3"][:].unsqueeze(1))

        # ---- adjacency -> replicated mask RA, both graphs up front so
        # graph 1's 4MB broadcast DMA completes long before it is needed ----
        ras = []
        for g in range(GPC):
            af = gbuf.tile([128, 128], F32, tag="af")
            nc.vector.tensor_copy(af[:], aiin[g][:])      # int32 -> f32
            atp = spsum.tile([128, 128], F32, tag="sp")
            nc.tensor.transpose(atp[:], af[:], ident[:])  # AT[j,i] in PSUM
            atb = gbuf.tile([128, 128], BF16, tag="atb")
            nc.scalar.copy(atb[:], atp[:])
            atd = dpool.tile([128, 128], BF16, tag=f"atd{g}")
            nc.scalar.dma_start(atd[:], atb[:])
            # reorder AT[j,i] into mask-flat order f = (c1, t, p) in DRAM
            mfd = dpool.tile([FREE], BF16, tag=f"mfd{g}")
            nc.scalar.dma_start(
                mfd[:].rearrange("(c t p) -> c t p", c=8, t=16),
                atd[:].rearrange("(t c) p -> c t p", c=8))
            # replicate to all 128 partitions with a 0-step broadcast DMA
            ra = gbuf.tile([128, FREE], BF16, tag=f"ra{g}")
            nc.scalar.dma_start(ra[:], mfd[:].unsqueeze(0).to_broadcast([128, FREE]))
            ras.append(ra)

        # ---------------- per-graph pipeline ----------------
        for g in range(GPC):
            ra = ras[g]

            # ---- e0 transpose: E0T[(c1,k), (t,p)] bf16 ----
            e0t = gbuf.tile([128, N * DE0], BF16, tag="e0t")
            for q in range(4):
                tp = zpool.tile([128, CH], F32, tag="z")
                for r in range(4):
                    t16 = 4 * q + r
                    nc.tensor.transpose(
                        tp[:, r * 128:(r + 1) * 128],
                        e0nat[g][:, 128 * t16:128 * (t16 + 1)], ident[:])
                nc.scalar.copy(e0t[:, q * CH:(q + 1) * CH], tp[:])

            # ---- x0T [c,i] bf16 ----
            x0tp = spsum.tile([128, 128], F32, tag="sp")
            nc.tensor.transpose(x0tp[0:DN0, :], x0in[g][:], ident[:])
            x0T = gbuf.tile([DN0, 128], BF16, tag="x0T")
            nc.scalar.copy(x0T[:], x0tp[0:DN0, :])

            msg0 = gbuf.tile([128, FREE], BF16, tag="msg0")
            msg1 = gbuf.tile([128, FREE], BF16, tag="msg1")
            bufA, bufB = (msg0, msg1) if g % 2 == 0 else (msg1, msg0)
            scratch = gbuf.tile([128, FREE // 4], BF16, tag="scratch")
            hsum = gbuf.tile([128, NCH], F32, tag="hsum")

            xT = x0T
            for layer in range(3):
                Wi, Wj = w[f"Wi{layer}"], w[f"Wj{layer}"]
                We = None if layer == 0 else w[f"We{layer}"]
                # xi' = xT.T @ Wi + b ; xj' = xT.T @ Wj   ([i,d] / [j,d])
                xip = spsum.tile([128, 128], F32, tag="sp")
                nc.tensor.matmul(xip[:], xT[:], Wi[:], start=True, stop=True)
                xib = small.tile([128, 128], BF16, tag="xib")
                nc.vector.tensor_add(xib[:], xip[:], brep[layer][:])
                xjp = spsum.tile([128, 128], F32, tag="sp")
                nc.tensor.matmul(xjp[:], xT[:], Wj[:], start=True, stop=True)
                xjb = small.tile([128, 128], BF16, tag="xjb")
                nc.scalar.copy(xjb[:], xjp[:])

                rhs_e = bufA if layer else None
                dst = bufA if layer == 0 else (bufB if layer == 1 else None)
                q4 = FREE // 4

                for k in range(NCH):
                    cols = slice(k * CH, (k + 1) * CH)
                    z = zpool.tile([128, CH], F32, tag="z")
                    if layer == 0:
                        c1, t4 = divmod(k, 4)
                        nc.tensor.matmul(
                            z[:], we0blk[c1][:],
                            e0t[:, t4 * CH:(t4 + 1) * CH],
                            start=True, stop=False)
                        cols = slice(c1 * 2048 + t4 * CH, c1 * 2048 + (t4 + 1) * CH)
                    else:
                        nc.tensor.matmul(z[:], We[:], rhs_e[:, cols],
                                         start=True, stop=False)
                    nc.tensor.matmul(z[:], xib[:], seli[:], start=False, stop=False)
                    nc.tensor.matmul(z[:], xjb[:], seljm[:, cols],
                                     start=False, stop=True)
                    # relu-evict on ACT, mask on DVE (accum_out = readout on L2)
                    if layer == 2:
                        # relu+mask+readout-accum in one in-place PSUM op;
                        # msg2 itself is never materialized
                        nc.vector.scalar_tensor_tensor(
                            out=z[:], in0=z[:], scalar=0.0, in1=ra[:, cols],
                            op0=OP.max, op1=OP.mult, accum_out=hsum[:, k:k + 1])
                    else:
                        raw = dst[:, cols]
                        nc.scalar.activation(raw, z[:], AF.Relu)
                        nc.vector.tensor_mul(raw, raw, ra[:, cols])

                if layer < 2:
                    # agg tree over (c1,t): 16384 -> 128, then node update
                    src = dst
                    nc.vector.tensor_add(scratch[:, 0:q4], src[:, 0:q4],
                                         src[:, q4:2 * q4])
                    nc.vector.tensor_add(scratch[:, 0:q4], scratch[:, 0:q4],
                                         src[:, 2 * q4:3 * q4])
                    nc.vector.tensor_add(scratch[:, 0:q4], scratch[:, 0:q4],
                                         src[:, 3 * q4:4 * q4])
                    width = q4
                    while width > 128:
                        h = width // 2
                        nc.vector.tensor_add(scratch[:, 0:h], scratch[:, 0:h],
                                             scratch[:, h:width])
                        width = h
                    aggT = small.tile([128, 128], BF16, tag="aggT")
                    nc.vector.tensor_copy(aggT[:], scratch[:, 0:128])

                    Wx, Wa = w[f"Wx{layer}"], w[f"Wa{layer}"]
                    xnp = spsum.tile([128, 128], F32, tag="sp")
                    nc.tensor.matmul(xnp[:], Wx[:], xT[:], start=True, stop=False)
                    nc.tensor.matmul(xnp[:], Wa[:], aggT[:], start=False, stop=True)
                    xnT = small.tile([128, 128], BF16, tag="xnT")
                    nc.scalar.activation(xnT[:], xnp[:], AF.Relu,
                                         bias=bcol[f"b_node_{layer}"][:])
                    if layer == 1:
                        # x-residual (x1+x2); the 0.5 is folded into Wi2/Wj2
                        xbl = small.tile([128, 128], BF16, tag="xbl")
                        nc.vector.tensor_add(xbl[:], xnT[:], xT[:])
                        xT = xbl
                    else:
                        xT = xnT

                if layer == 1:
                    # e-blend (bufA+bufB) -> bufA; the 0.5 is folded into We2
                    for k in range(NCH):
                        cols = slice(k * CH, (k + 1) * CH)
                        nc.vector.tensor_add(bufA[:, cols], bufA[:, cols],
                                             bufB[:, cols])

            # ---- readout head ----
            hpre = small.tile([128, 1], F32, tag="hpre")
            nc.vector.tensor_reduce(hpre[:], hsum[:], axis=mybir.AxisListType.X,
                                    op=OP.add)
            h1p = spsum.tile([128, 128], F32, tag="sp")
            nc.tensor.matmul(h1p[:, 0:1], wh1[:], hpre[:], start=True, stop=True)
            h1 = small.tile([128, 1], F32, tag="h1")
            nc.scalar.activation(h1[:], h1p[:, 0:1], AF.Relu,
                                 bias=bcol["b_h1"][:], scale=1.0 / FREE)
            h2p = spsum.tile([128, 128], F32, tag="sp")
            nc.tensor.matmul(h2p[:, 0:1], wh2[:], h1[:], start=True, stop=True)
            h2 = small.tile([128, 1], F32, tag="h2")
            nc.scalar.activation(h2[:], h2p[:, 0:1], AF.Relu, bias=bcol["b_h2"][:])
            h3p = spsum.tile([128, 128], F32, tag="sp")
            nc.tensor.matmul(h3p[0:1, 0:1], wh3[:], h2[:], start=True, stop=True)
            oval = small.tile([1, 1], F32, tag="oval")
            nc.scalar.activation(oval[:], h3p[0:1, 0:1], AF.Identity, bias=bh3[:])
            nc.sync.dma_start(out_d[g:g + 1, :], oval[:])

        stack.close()
    nc.finalize()
    return nc


def kernel(**inputs):
    inputs = {k: np.asarray(v) for k, v in inputs.items()}
    if "nc" not in _CACHE:
        _CACHE["nc"] = build_nc()
    nc = _CACHE["nc"]

    in_maps = []
    for c in range(NCORES):
        m = {
            "edge_index": np.ascontiguousarray(inputs["edge_index"][c * GPC:(c + 1) * GPC]),
            "x": np.ascontiguousarray(inputs["x"][c * GPC:(c + 1) * GPC]),
            "edge_attr": np.ascontiguousarray(inputs["edge_attr"][c * GPC:(c + 1) * GPC]),
        }
        for n_ in WEIGHT_NAMES:
            m[n_] = np.ascontiguousarray(inputs[n_], dtype=np.float32)
        in_maps.append(m)

    from concourse.bass_utils import run_bass_kernel_spmd
    res = run_bass_kernel_spmd(nc, in_maps, list(range(NCORES)))
    out = np.concatenate([np.asarray(res.results[c]["out"]).reshape(-1)
                          for c in range(NCORES)])
    return out.astype(np.float32)



# revision 8
# speedup vs baseline: 1.9062x; 1.9062x over previous
"""Trainium2 Bass kernel for nn_Disc_edge2 (3-layer dense-graph GNN + MLP head).

Sharding: data-parallel over batch B=16 across 8 cores (2 graphs/core).

Per-graph msg layout: [do=128 partitions, f=16384] with f = c1*2048 + t*128 + p,
edge (i, j) -> p = i, j = 8*t + c1.

All heavy compute runs as fp8e4m3 DoubleRow matmuls (2 k-tiles per pass, 0.5
cycles/row). Per 512-col chunk, layers 0/1 need just TWO DoubleRow matmuls:
    DR1: (We   @ e-chunk)   + (xib @ seli)     e-term + xi broadcast
    DR2: (xjb  @ seljm)     + (mstat @ maskA)  xj broadcast + adjacency mask
The adjacency mask is folded into the PSUM accumulation as -960*(1-A[f]) so the
relu eviction zeroes non-edges for free; no tensor-tensor mask pass exists.
Layer 2 adds a third DR for the residual e-blend (msg0@We2' + msg1@We2', with
the 0.5 folded into We2') and accumulates the edge-mean readout via accum_out
on the eviction op; msg2 is never materialized.

The j-aggregation agg@Wa runs on PE as 128 accumulating DoubleRow matmuls over
j-blocks with a two-digit fp8 decomposition of Wa (hi+lo), giving ~bf16
accuracy at fp8 speed and directly producing the transposed node update.

Evictions (PSUM->SBUF relu, the only remaining elementwise work) round-robin
across ACT / DVE / Pool in [128,1024] two-bank ops.

Weight-derived constants, selection matrices (seli/seljm/maskA) and the
transposed fp8 edge_attr are laid out host-side; the two operand "arenas" are
single SBUF tiles so DoubleRow k-tile pairs can be addressed by inserting a
[stride, 2] dim into the access patterns.
"""

import sys

sys.path.insert(0, "/opt/trn_rl_repo")

import numpy as np
import ml_dtypes

import concourse.bass as bass
from concourse import bacc
import concourse.mybir as mybir
import concourse.tile as tile

F32 = mybir.dt.float32
BF16 = mybir.dt.bfloat16
F8 = mybir.dt.float8e4
AF = mybir.ActivationFunctionType
OP = mybir.AluOpType
DR = mybir.MatmulPerfMode.DoubleRow

NPF8 = ml_dtypes.float8_e4m3
NPBF = ml_dtypes.bfloat16

B, N, DN0, DE0, DH = 16, 128, 64, 16, 128
NCORES = 8
GPC = B // NCORES
FREE = N * N              # 16384
CH = 512
NCH = FREE // CH          # 32 chunks
NPAIR = NCH // 2          # 16 chunk-pairs ([128,1024] evictions)

# ---- moving arena (fp8) column offsets ----
# ISA pattern steps are 16-bit (+-32767 elements), so each graph's L1 msg->seli
# k-tile pair needs a seli copy within 32K columns: seli (g0) + seli_b (g1).
O_SELJM = 0
O_MASKA = O_SELJM + FREE          # 16384
O_SELI = O_MASKA + FREE           # 32768
O_E0T = [O_SELI + CH, O_SELI + CH + 2048]        # per graph
O_MSG = [[O_E0T[1] + 2048, O_E0T[1] + 2048 + FREE],
         [O_E0T[1] + 2048 + 2 * FREE + CH, O_E0T[1] + 2048 + 3 * FREE + CH]]
O_SELI_B = O_MSG[0][1] + FREE     # second seli copy, just before msg0_g1
O_SCR = O_MSG[1][1] + FREE        # 2 x 1024 scratch (ACT/DVE)
MV_TOTAL = O_SCR + 2 * 1024

# ---- stationary arena (fp8) column offsets ----
# [dyn g0 | consts | dyn g1]; dyn = xib/xjb per layer
S_DYN = [0, None]
S_WE0 = 768                       # 8 x 128 block-diag variants
S_WE1 = S_WE0 + 1024
S_WE2H2 = S_WE1 + 128             # [0.5*We2 | 0.5*We2]
S_WAHI = [S_WE2H2 + 256, S_WE2H2 + 256 + 512]   # per layer: hi2(256)+lo2(256)
S_MSTAT = S_WAHI[1] + 512         # per graph 128
S_ZERO = S_MSTAT + 256
S_DYN[1] = S_ZERO + 128
ST_TOTAL = S_DYN[1] + 768

WEIGHT_NAMES = [
    "w_msg_0", "b_msg_0", "w_node_0", "b_node_0",
    "w_msg_1", "b_msg_1", "w_node_1", "b_node_1",
    "w_msg_2", "b_msg_2", "w_node_2", "b_node_2",
    "w_h1", "b_h1", "w_h2", "b_h2", "w_h3", "b_h3",
]

_CACHE = {}


def _pair(ap_a, ap_b):
    """AP reading ap_a as k-tile slot 0 and ap_b as slot 1 (inserted dim)."""
    c = ap_a.copy()
    c.ap.insert(1, [ap_b.offset - ap_a.offset, 2])
    return c


def build_nc():
    nc = bacc.Bacc()

    mv_d = nc.declare_dram_parameter("mvconst", [128, O_E0T[0]], F8, isOutput=False)
    st_d = nc.declare_dram_parameter("stconst", [128, S_DYN[1] - S_WE0], F8,
                                     isOutput=False)
    e0t_d = nc.declare_dram_parameter("e0t", [GPC, 128, 2048], F8, isOutput=False)
    xt_d = nc.declare_dram_parameter("xt", [GPC, DN0, 128], BF16, isOutput=False)
    wbf_d = nc.declare_dram_parameter("wbf", [128, 1024], BF16, isOutput=False)
    brep_d = nc.declare_dram_parameter("brep", [128, 384], F32, isOutput=False)
    bcol_d = nc.declare_dram_parameter("bcol", [128, 4], F32, isOutput=False)
    whd_d = nc.declare_dram_parameter("whd", [128, 257], F32, isOutput=False)
    bh3_d = nc.declare_dram_parameter("bh3", [1, 1], F32, isOutput=False)
    out_d = nc.declare_dram_parameter("out", [GPC, 1], F32, isOutput=True)

    with tile.TileContext(nc) as tc:
        import contextlib
        stack = contextlib.ExitStack()
        gbuf = stack.enter_context(tc.tile_pool(name="gbuf", bufs=1))
        small = stack.enter_context(tc.tile_pool(name="small", bufs=2))
        zpool = stack.enter_context(tc.tile_pool(name="zp", bufs=2, space="PSUM"))
        xpool = stack.enter_context(tc.tile_pool(name="xp", bufs=2, space="PSUM"))
        npool = stack.enter_context(tc.tile_pool(name="np", bufs=2, space="PSUM"))

        mva = gbuf.tile([128, MV_TOTAL], F8, tag="mva")
        sta = gbuf.tile([128, ST_TOTAL], F8, tag="sta")
        wbf = gbuf.tile([128, 1024], BF16, tag="wbf")
        brep = gbuf.tile([128, 384], F32, tag="brep")
        bcol = gbuf.tile([128, 4], F32, tag="bcol")
        whd = gbuf.tile([128, 257], F32, tag="whd")
        bh3 = gbuf.tile([1, 1], F32, tag="bh3")
        xt0 = [gbuf.tile([DN0, 128], BF16, tag=f"xt0_{g}", name=f"xt0_{g}")
               for g in range(GPC)]
        hsum = [gbuf.tile([128, NPAIR], F32, tag=f"hs{g}", name=f"hs{g}")
                for g in range(GPC)]

        # ---- small loads on the scalar queue ----
        nc.scalar.dma_start(sta[:, S_WE0:S_DYN[1]], st_d[:, :])
        nc.scalar.dma_start(wbf[:], wbf_d[:, :])
        nc.scalar.dma_start(brep[:], brep_d[:, :])
        nc.scalar.dma_start(bcol[:], bcol_d[:, :])
        nc.scalar.dma_start(whd[:], whd_d[:, :])
        nc.scalar.dma_start(bh3[:], bh3_d[:, :])
        for g in range(GPC):
            nc.scalar.dma_start(xt0[g][:], xt_d[g])

        # ---- big loads on the sync queue, in consumption order ----
        nc.sync.dma_start(mva[:, O_SELI:O_SELI + CH],
                          mv_d[:, O_SELI:O_SELI + CH])
        nc.scalar.dma_start(mva[:, O_SELI_B:O_SELI_B + CH],
                            mv_d[:, O_SELI:O_SELI + CH])
        for g in range(GPC):
            nc.sync.dma_start(mva[:, O_E0T[g]:O_E0T[g] + 2048],
                              e0t_d[g].rearrange("p f -> p f"))
        # seljm + maskA interleaved per c1-block so chunk 0 can start early
        for c1 in range(8):
            a, b = c1 * 2048, (c1 + 1) * 2048
            nc.sync.dma_start(mva[:, O_SELJM + a:O_SELJM + b],
                              mv_d[:, O_SELJM + a:O_SELJM + b])
            nc.sync.dma_start(mva[:, O_MASKA + a:O_MASKA + b],
                              mv_d[:, O_MASKA + a:O_MASKA + b])

        seli_ap = mva[:, O_SELI:O_SELI + CH]
        xTs = [xt0[g] for g in range(GPC)]   # updated per layer

        def msg_layer(g, l):
            """Emit chunk DRs + evictions for layer l of graph g."""
            dyn = S_DYN[g]
            xib = sta[:, dyn + l * 256:dyn + l * 256 + 128]
            xjb = sta[:, dyn + l * 256 + 128:dyn + l * 256 + 256]
            mstat = sta[:, S_MSTAT + g * 128:S_MSTAT + (g + 1) * 128]
            zero = sta[:, S_ZERO:S_ZERO + 128]
            msg_off = O_MSG[g][l] if l < 2 else None
            for cp in range(NPAIR):
                z = zpool.tile([128, 1024], F32, tag="z")
                for h in range(2):
                    k = 2 * cp + h
                    zz = z[:, h * 512:(h + 1) * 512]
                    sjm = mva[:, O_SELJM + k * CH:O_SELJM + (k + 1) * CH]
                    mka = mva[:, O_MASKA + k * CH:O_MASKA + (k + 1) * CH]
                    if l == 0:
                        c1, t4 = divmod(k, 4)
                        emv = mva[:, O_E0T[g] + t4 * CH:O_E0T[g] + (t4 + 1) * CH]
                        est = sta[:, S_WE0 + c1 * 128:S_WE0 + (c1 + 1) * 128]
                        nc.tensor.matmul(zz, _pair(est, xib), _pair(emv, seli_ap),
                                         start=True, stop=False, perf_mode=DR)
                    elif l == 1:
                        po = O_MSG[g][0]
                        emv = mva[:, po + k * CH:po + (k + 1) * CH]
                        est = sta[:, S_WE1:S_WE1 + 128]
                        sel = seli_ap if g == 0 else \
                            mva[:, O_SELI_B:O_SELI_B + CH]
                        nc.tensor.matmul(zz, _pair(est, xib), _pair(emv, sel),
                                         start=True, stop=False, perf_mode=DR)
                    else:
                        p0, p1 = O_MSG[g][0], O_MSG[g][1]
                        we2 = sta[:, S_WE2H2:S_WE2H2 + 256].rearrange(
                            "p (two m) -> p two m", two=2)
                        nc.tensor.matmul(
                            zz, we2,
                            _pair(mva[:, p0 + k * CH:p0 + (k + 1) * CH],
                                  mva[:, p1 + k * CH:p1 + (k + 1) * CH]),
                            start=True, stop=False, perf_mode=DR)
                        nc.tensor.matmul(zz, _pair(xib, xjb),
                                         _pair(seli_ap, sjm),
                                         start=False, stop=False, perf_mode=DR)
                        nc.tensor.matmul(zz, _pair(mstat, zero),
                                         _pair(mka, sjm),
                                         start=False, stop=True, perf_mode=DR)
                    if l < 2:
                        nc.tensor.matmul(zz, _pair(xjb, mstat), _pair(sjm, mka),
                                         start=False, stop=True, perf_mode=DR)
                # evict the chunk-pair: relu (+mask already in PSUM).
                # GPSIMD cannot read PSUM, so only ACT/DVE evict (9:7 split).
                on_act = cp % 2 == 0 or cp == 15
                if l < 2:
                    dst = mva[:, msg_off + cp * 1024:msg_off + (cp + 1) * 1024]
                    if on_act:
                        nc.scalar.activation(dst, z[:], AF.Relu)
                    else:
                        nc.vector.tensor_scalar(dst, z[:], 0.0, None, OP.max)
                else:
                    acc = hsum[g][:, cp:cp + 1]
                    if on_act:
                        scr = mva[:, O_SCR:O_SCR + 1024]
                        nc.scalar.activation(scr, z[:], AF.Relu, accum_out=acc)
                    else:
                        scr = mva[:, O_SCR + 1024:O_SCR + 2048]
                        nc.vector.tensor_scalar(scr, z[:], 0.0, None, OP.max,
                                                op1=OP.add, accum_out=acc)

        def xi_xj(g, l):
            """xi' = x@Wi + b, xj' = x@Wj as fp8 stationaries in the arena."""
            xT = xTs[g]
            K = DN0 if l == 0 else DH
            wcol = l * 384 if l < 2 else 768
            ps = xpool.tile([128, 256], F32, tag="xixj")
            nc.tensor.matmul(ps[:, 0:128], xT[:], wbf[0:K, wcol:wcol + 128],
                             start=True, stop=True)
            nc.tensor.matmul(ps[:, 128:256], xT[:], wbf[0:K, wcol + 128:wcol + 256],
                             start=True, stop=True)
            dyn = S_DYN[g]
            nc.vector.tensor_tensor(sta[:, dyn + l * 256:dyn + l * 256 + 128],
                                    ps[:, 0:128], brep[:, l * 128:(l + 1) * 128],
                                    op=OP.add)
            nc.vector.tensor_copy(sta[:, dyn + l * 256 + 128:dyn + l * 256 + 256],
                                  ps[:, 128:256])

        def node_update(g, l):
            """x_new^T = relu(Wx^T x^T + Wa^T agg^T + b) on PE via j-block DRs."""
            xT = xTs[g]
            K = DN0 if l == 0 else DH
            wcol = l * 384 + 256
            xn = npool.tile([128, 128], F32, tag="xn")
            nc.tensor.matmul(xn[:], wbf[0:K, wcol:wcol + 128], xT[:],
                             start=True, stop=False)
            wa = sta[:, S_WAHI[l]:S_WAHI[l] + 256].rearrange(
                "p (two m) -> p two m", two=2)
            wl = sta[:, S_WAHI[l] + 256:S_WAHI[l] + 512].rearrange(
                "p (two m) -> p two m", two=2)
            mo = O_MSG[g][l]
            for b2 in range(64):
                mb = mva[:, mo + b2 * 256:mo + (b2 + 1) * 256].rearrange(
                    "p (two m) -> p two m", two=2)
                nc.tensor.matmul(xn[:], wa, mb, start=False, stop=False,
                                 perf_mode=DR)
                nc.tensor.matmul(xn[:], wl, mb, start=False, stop=(b2 == 63),
                                 perf_mode=DR)
            xnT = small.tile([128, 128], BF16, tag=f"xnT{g}_{l}")
            nc.scalar.activation(xnT[:], xn[:], AF.Relu, bias=bcol[:, l:l + 1])
            if l == 1:
                xbl = small.tile([128, 128], BF16, tag=f"xbl{g}")
                nc.vector.tensor_add(xbl[:], xnT[:], xTs[g][:])
                xTs[g] = xbl
            else:
                xTs[g] = xnT

        def head(g):
            hpre = small.tile([128, 1], F32, tag=f"hp{g}")
            nc.vector.tensor_reduce(hpre[:], hsum[g][:], axis=mybir.AxisListType.X,
                                    op=OP.add)
            h1p = npool.tile([128, 128], F32, tag="xn")
            nc.tensor.matmul(h1p[:, 0:1], whd[:, 0:128], hpre[:],
                             start=True, stop=True)
            h1 = small.tile([128, 1], F32, tag=f"h1{g}")
            nc.scalar.activation(h1[:], h1p[:, 0:1], AF.Relu,
                                 bias=bcol[:, 2:3], scale=1.0 / FREE)
            h2p = npool.tile([128, 128], F32, tag="xn")
            nc.tensor.matmul(h2p[:, 0:1], whd[:, 128:256], h1[:],
                             start=True, stop=True)
            h2 = small.tile([128, 1], F32, tag=f"h2{g}")
            nc.scalar.activation(h2[:], h2p[:, 0:1], AF.Relu, bias=bcol[:, 3:4])
            h3p = npool.tile([128, 128], F32, tag="xn")
            nc.tensor.matmul(h3p[0:1, 0:1], whd[:, 256:257], h2[:],
                             start=True, stop=True)
            oval = small.tile([1, 1], F32, tag=f"ov{g}")
            nc.scalar.activation(oval[:], h3p[0:1, 0:1], AF.Identity, bias=bh3[:])
            nc.sync.dma_start(out_d[g:g + 1, :], oval[:])

        # ---- schedule: layer-interleaved across the two graphs ----
        for l in range(3):
            for g in range(GPC):
                xi_xj(g, l)
                msg_layer(g, l)
            if l < 2:
                for g in range(GPC):
                    node_update(g, l)
        for g in range(GPC):
            head(g)

        stack.close()
    nc.finalize()
    return nc


def _f8(x):
    return np.asarray(x, dtype=np.float32).astype(NPF8)


def prep_core_inputs(inputs, core):
    """Host-side layout/dtype prep for one core's GPC graphs."""
    gs = slice(core * GPC, (core + 1) * GPC)
    A = np.asarray(inputs["edge_index"][gs], np.float32)        # [GPC,N,N]
    x = np.asarray(inputs["x"][gs], np.float32)                 # [GPC,N,DN0]
    ea = np.asarray(inputs["edge_attr"][gs], np.float32)        # [GPC,N,N,DE0]
    w = {k: np.asarray(inputs[k], np.float32) for k in WEIGHT_NAMES}

    f = np.arange(FREE)
    c1f, tf, pf = f // 2048, (f // 128) % 16, f % 128
    jf = 8 * tf + c1f

    # moving constants: seljm | maskA | seli
    mv = np.zeros((128, O_E0T[0]), NPF8)
    mv[:, O_SELJM:O_SELJM + FREE] = \
        (np.arange(128)[:, None] == jf[None, :]).astype(NPF8)
    for g in range(GPC):
        mv[g, O_MASKA:O_MASKA + FREE] = \
            (4.0 * (1.0 - A[g][pf, jf])).astype(NPF8)
    seli = (np.arange(128)[:, None] == (np.arange(CH) % 128)[None, :])
    mv[:, O_SELI:O_SELI + CH] = seli.astype(NPF8)

    # stationary constants
    st = np.zeros((128, S_DYN[1] - S_WE0), NPF8)
    o = -S_WE0
    We0 = w["w_msg_0"][2 * DN0:]                                # [16,128]
    for c1 in range(8):
        st[16 * c1:16 * (c1 + 1), o + S_WE0 + c1 * 128:o + S_WE0 + (c1 + 1) * 128] \
            = _f8(We0)
    st[:, o + S_WE1:o + S_WE1 + 128] = _f8(w["w_msg_1"][2 * DH:])
    we2h = _f8(0.5 * w["w_msg_2"][2 * DH:])
    st[:, o + S_WE2H2:o + S_WE2H2 + 128] = we2h
    st[:, o + S_WE2H2 + 128:o + S_WE2H2 + 256] = we2h
    for l in range(2):
        Dn = DN0 if l == 0 else DH
        Wa = w[f"w_node_{l}"][Dn:]
        hi = _f8(Wa)
        lo = _f8(Wa - hi.astype(np.float32))
        base = o + S_WAHI[l]
        st[:, base:base + 128] = hi
        st[:, base + 128:base + 256] = hi
        st[:, base + 256:base + 384] = lo
        st[:, base + 384:base + 512] = lo
    for g in range(GPC):
        st[g, o + S_MSTAT + g * 128:o + S_MSTAT + (g + 1) * 128] = \
            np.asarray(-240.0, NPF8)

    # transposed fp8 edge features: [(j8,de), (t16,i)]
    e0t = np.ascontiguousarray(
        ea.reshape(GPC, N, 16, 8, DE0).transpose(0, 3, 4, 2, 1)
    ).reshape(GPC, 128, 2048).astype(NPF8)

    xt = np.ascontiguousarray(x.transpose(0, 2, 1)).astype(NPBF)

    # bf16 x-path weights: per layer [Wi | Wj | Wx], L2 has 0.5-folded Wi/Wj
    wbf = np.zeros((128, 1024), NPBF)
    for l in range(2):
        Dn = DN0 if l == 0 else DH
        wm, wn = w[f"w_msg_{l}"], w[f"w_node_{l}"]
        wbf[0:Dn, l * 384:l * 384 + 128] = wm[0:Dn].astype(NPBF)
        wbf[0:Dn, l * 384 + 128:l * 384 + 256] = wm[Dn:2 * Dn].astype(NPBF)
        wbf[0:Dn, l * 384 + 256:l * 384 + 384] = wn[0:Dn].astype(NPBF)
    wbf[0:DH, 768:896] = (0.5 * w["w_msg_2"][0:DH]).astype(NPBF)
    wbf[0:DH, 896:1024] = (0.5 * w["w_msg_2"][DH:2 * DH]).astype(NPBF)

    brep = np.zeros((128, 384), np.float32)
    for l in range(3):
        brep[:, l * 128:(l + 1) * 128] = w[f"b_msg_{l}"][None, :]
    bcol = np.stack([w["b_node_0"], w["b_node_1"], w["b_h1"], w["b_h2"]],
                    axis=1).astype(np.float32)
    whd = np.zeros((128, 257), np.float32)
    whd[:, 0:128] = w["w_h1"]
    whd[:, 128:256] = w["w_h2"]
    whd[:, 256:257] = w["w_h3"]
    bh3 = w["b_h3"].reshape(1, 1).astype(np.float32)

    return {
        "mvconst": mv, "stconst": st, "e0t": e0t, "xt": xt, "wbf": wbf,
        "brep": brep, "bcol": bcol, "whd": whd, "bh3": bh3,
    }


def kernel(**inputs):
    inputs = {k: np.asarray(v) for k, v in inputs.items()}
    if "nc" not in _CACHE:
        _CACHE["nc"] = build_nc()
    nc = _CACHE["nc"]

    in_maps = [prep_core_inputs(inputs, c) for c in range(NCORES)]

    from concourse.bass_utils import run_bass_kernel_spmd
    res = run_bass_kernel_spmd(nc, in_maps, list(range(NCORES)))
    out = np.concatenate([np.asarray(res.results[c]["out"]).reshape(-1)
                          for c in range(NCORES)])
    return out.astype(np.float32)


# revision 13
# speedup vs baseline: 2.3908x; 1.2542x over previous
"""Trainium2 Bass kernel for nn_Disc_edge2 (3-layer dense-graph GNN + MLP head).

Sharding: data-parallel over batch B=16 across 8 cores (2 graphs/core).

Per-graph msg layout: [do=128 partitions, f=16384] with f = c1*2048 + t*128 + p,
edge (i, j) -> p = i, j = 8*t + c1.

All heavy compute runs as fp8e4m3 DoubleRow matmuls (2 k-tiles per pass, 0.5
cycles/row). Per 512-col chunk, layers 0/1 need just TWO DoubleRow matmuls:
    DR1: (We   @ e-chunk)   + (xib @ seli)     e-term + xi broadcast
    DR2: (xjb  @ seljm)     + (mstat @ maskA)  xj broadcast + adjacency mask
The adjacency mask is folded into the PSUM accumulation as -960*(1-A[f]) so the
relu eviction zeroes non-edges for free; no tensor-tensor mask pass exists.
Layer 2 adds a third DR for the residual e-blend (msg0@We2' + msg1@We2', with
the 0.5 folded into We2') and accumulates the edge-mean readout via accum_out
on the eviction op; msg2 is never materialized.

The j-aggregation agg@Wa runs on PE as 128 accumulating DoubleRow matmuls over
j-blocks with a two-digit fp8 decomposition of Wa (hi+lo), giving ~bf16
accuracy at fp8 speed and directly producing the transposed node update.

Evictions (PSUM->SBUF relu, the only remaining elementwise work) round-robin
across ACT / DVE / Pool in [128,1024] two-bank ops.

Weight-derived constants, selection matrices (seli/seljm/maskA) and the
transposed fp8 edge_attr are laid out host-side; the two operand "arenas" are
single SBUF tiles so DoubleRow k-tile pairs can be addressed by inserting a
[stride, 2] dim into the access patterns.
"""

import sys

sys.path.insert(0, "/opt/trn_rl_repo")

import numpy as np
import ml_dtypes

import concourse.bass as bass
from concourse import bacc
import concourse.mybir as mybir
import concourse.tile as tile

F32 = mybir.dt.float32
BF16 = mybir.dt.bfloat16
F8 = mybir.dt.float8e4
AF = mybir.ActivationFunctionType
OP = mybir.AluOpType
DR = mybir.MatmulPerfMode.DoubleRow

NPF8 = ml_dtypes.float8_e4m3
NPBF = ml_dtypes.bfloat16

B, N, DN0, DE0, DH = 16, 128, 64, 16, 128
NCORES = 8
GPC = B // NCORES
FREE = N * N              # 16384
CH = 512
NCH = FREE // CH          # 32 chunks
NPAIR = NCH // 2          # 16 chunk-pairs ([128,1024] evictions)

# ---- moving arena (fp8) column offsets ----
# ISA pattern steps are 16-bit (+-32767 elements), so each graph's L1 msg->seli
# k-tile pair needs a seli copy within 32K columns: seli (g0) + seli_b (g1).
O_SELJM = 0
O_MASKA = O_SELJM + FREE          # 16384
O_SELI = O_MASKA + FREE           # 32768
O_E0T = [O_SELI + CH, O_SELI + CH + 2048]        # per graph
O_MSG = [[O_E0T[1] + 2048, O_E0T[1] + 2048 + FREE],
         [O_E0T[1] + 2048 + 2 * FREE + CH, O_E0T[1] + 2048 + 3 * FREE + CH]]
O_SELI_B = O_MSG[0][1] + FREE     # second seli copy, just before msg0_g1
O_SCR = O_MSG[1][1] + FREE        # 2 x 1024 scratch (ACT/DVE)
MV_TOTAL = O_SCR + 2 * 1024

# ---- stationary arena (fp8) column offsets ----
# [dyn g0 | consts | dyn g1]; dyn = xib/xjb per layer
S_DYN = [0, None]
S_WE0 = 768                       # 8 x 128 block-diag variants
S_WE1 = S_WE0 + 1024
S_WE2H2 = S_WE1 + 128             # [0.5*We2 | 0.5*We2]
S_WAHI = [S_WE2H2 + 256, S_WE2H2 + 256 + 512]   # per layer: hi2(256)+lo2(256)
S_MSTAT = S_WAHI[1] + 512         # per graph 128
S_ZERO = S_MSTAT + 256
S_DYN[1] = S_ZERO + 128
ST_TOTAL = S_DYN[1] + 768

WEIGHT_NAMES = [
    "w_msg_0", "b_msg_0", "w_node_0", "b_node_0",
    "w_msg_1", "b_msg_1", "w_node_1", "b_node_1",
    "w_msg_2", "b_msg_2", "w_node_2", "b_node_2",
    "w_h1", "b_h1", "w_h2", "b_h2", "w_h3", "b_h3",
]

_CACHE = {}


def _pair(ap_a, ap_b):
    """AP reading ap_a as k-tile slot 0 and ap_b as slot 1 (inserted dim)."""
    c = ap_a.copy()
    c.ap.insert(1, [ap_b.offset - ap_a.offset, 2])
    return c


def build_nc():
    nc = bacc.Bacc()

    mv_d = nc.declare_dram_parameter("mvconst", [128, FREE + CH], F8,
                                     isOutput=False)
    marow_d = nc.declare_dram_parameter("marow", [GPC, FREE], F8, isOutput=False)
    st_d = nc.declare_dram_parameter("stconst", [128, S_DYN[1] - S_WE0], F8,
                                     isOutput=False)
    e0t_d = nc.declare_dram_parameter("e0t", [GPC, 128, 2048], F8, isOutput=False)
    xt_d = nc.declare_dram_parameter("xt", [GPC, DN0, 128], BF16, isOutput=False)
    wbf_d = nc.declare_dram_parameter("wbf", [128, 1024], BF16, isOutput=False)
    brep_d = nc.declare_dram_parameter("brep", [128, 384], F32, isOutput=False)
    bcol_d = nc.declare_dram_parameter("bcol", [128, 4], F32, isOutput=False)
    whd_d = nc.declare_dram_parameter("whd", [128, 257], F32, isOutput=False)
    bh3_d = nc.declare_dram_parameter("bh3", [1, 1], F32, isOutput=False)
    out_d = nc.declare_dram_parameter("out", [GPC, 1], F32, isOutput=True)

    with tile.TileContext(nc) as tc:
        import contextlib
        stack = contextlib.ExitStack()
        gbuf = stack.enter_context(tc.tile_pool(name="gbuf", bufs=1))
        small = stack.enter_context(tc.tile_pool(name="small", bufs=2))
        zpool = stack.enter_context(tc.tile_pool(name="zp", bufs=3, space="PSUM"))
        xpool = stack.enter_context(tc.tile_pool(name="xp", bufs=1, space="PSUM"))
        npool = stack.enter_context(tc.tile_pool(name="np", bufs=1, space="PSUM"))

        mva = gbuf.tile([128, MV_TOTAL], F8, tag="mva")
        sta = gbuf.tile([128, ST_TOTAL], F8, tag="sta")
        wbf = gbuf.tile([128, 1024], BF16, tag="wbf")
        brep = gbuf.tile([128, 384], F32, tag="brep")
        bcol = gbuf.tile([128, 4], F32, tag="bcol")
        whd = gbuf.tile([128, 257], F32, tag="whd")
        bh3 = gbuf.tile([1, 1], F32, tag="bh3")
        xt0 = [gbuf.tile([DN0, 128], BF16, tag=f"xt0_{g}", name=f"xt0_{g}")
               for g in range(GPC)]
        hsum = [gbuf.tile([128, NPAIR], F32, tag=f"hs{g}", name=f"hs{g}")
                for g in range(GPC)]

        # ---- loads, ordered by first consumption (issue cost ~1.3us/DMA) ----
        # maskA zero rows come from a Pool memset (bitcast to f32 for 4-byte
        # lanes); only the two real notA rows are DMAd.
        nc.gpsimd.memset(mva[:, O_MASKA:O_MASKA + FREE].bitcast(F32), 0.0)
        # scalar queue: x-path deps first
        nc.scalar.dma_start(xt0[0][:], xt_d[0])
        nc.scalar.dma_start(wbf[:], wbf_d[:, :])
        nc.scalar.dma_start(brep[:], brep_d[:, :])
        nc.scalar.dma_start(sta[:, S_WE0:S_DYN[1]], st_d[:, :])
        for g in range(GPC):
            nc.scalar.dma_start(mva[g:g + 1, O_MASKA:O_MASKA + FREE],
                                marow_d[g:g + 1, :])
        nc.scalar.dma_start(xt0[1][:], xt_d[1])
        nc.scalar.dma_start(mva[:, O_SELI_B:O_SELI_B + CH],
                            mv_d[:, FREE:FREE + CH])
        nc.scalar.dma_start(bcol[:], bcol_d[:, :])
        nc.scalar.dma_start(whd[:], whd_d[:, :])
        nc.scalar.dma_start(bh3[:], bh3_d[:, :])
        # sync queue: seli, e0t, then seljm in two halves
        nc.sync.dma_start(mva[:, O_SELI:O_SELI + CH], mv_d[:, FREE:FREE + CH])
        nc.sync.dma_start(mva[:, O_E0T[0]:O_E0T[0] + 2048], e0t_d[0])
        nc.sync.dma_start(mva[:, O_SELJM:O_SELJM + 8192],
                          mv_d[:, 0:8192])
        nc.sync.dma_start(mva[:, O_E0T[1]:O_E0T[1] + 2048], e0t_d[1])
        nc.sync.dma_start(mva[:, O_SELJM + 8192:O_SELJM + FREE],
                          mv_d[:, 8192:FREE])

        seli_ap = mva[:, O_SELI:O_SELI + CH]
        xTs = [xt0[g] for g in range(GPC)]   # updated per layer

        def msg_layer(g, l):
            """Emit chunk DRs + evictions for layer l of graph g."""
            dyn = S_DYN[g]
            xib = sta[:, dyn + l * 256:dyn + l * 256 + 128]
            xjb = sta[:, dyn + l * 256 + 128:dyn + l * 256 + 256]
            mstat = sta[:, S_MSTAT + g * 128:S_MSTAT + (g + 1) * 128]
            zero = sta[:, S_ZERO:S_ZERO + 128]
            msg_off = O_MSG[g][l] if l < 2 else None
            for cp in range(NPAIR):
                z = zpool.tile([128, 1024], F32, tag="z")
                for h in range(2):
                    k = 2 * cp + h
                    zz = z[:, h * 512:(h + 1) * 512]
                    sjm = mva[:, O_SELJM + k * CH:O_SELJM + (k + 1) * CH]
                    mka = mva[:, O_MASKA + k * CH:O_MASKA + (k + 1) * CH]
                    if l == 0:
                        c1, t4 = divmod(k, 4)
                        emv = mva[:, O_E0T[g] + t4 * CH:O_E0T[g] + (t4 + 1) * CH]
                        est = sta[:, S_WE0 + c1 * 128:S_WE0 + (c1 + 1) * 128]
                        nc.tensor.matmul(zz, _pair(est, xib), _pair(emv, seli_ap),
                                         start=True, stop=False, perf_mode=DR)
                    elif l == 1:
                        po = O_MSG[g][0]
                        emv = mva[:, po + k * CH:po + (k + 1) * CH]
                        est = sta[:, S_WE1:S_WE1 + 128]
                        sel = seli_ap if g == 0 else \
                            mva[:, O_SELI_B:O_SELI_B + CH]
                        nc.tensor.matmul(zz, _pair(est, xib), _pair(emv, sel),
                                         start=True, stop=False, perf_mode=DR)
                    else:
                        p0, p1 = O_MSG[g][0], O_MSG[g][1]
                        we2 = sta[:, S_WE2H2:S_WE2H2 + 256].rearrange(
                            "p (two m) -> p two m", two=2)
                        nc.tensor.matmul(
                            zz, we2,
                            _pair(mva[:, p0 + k * CH:p0 + (k + 1) * CH],
                                  mva[:, p1 + k * CH:p1 + (k + 1) * CH]),
                            start=True, stop=False, perf_mode=DR)
                        nc.tensor.matmul(zz, _pair(xib, xjb),
                                         _pair(seli_ap, sjm),
                                         start=False, stop=False, perf_mode=DR)
                        nc.tensor.matmul(zz, _pair(mstat, zero),
                                         _pair(mka, sjm),
                                         start=False, stop=True, perf_mode=DR)
                    if l < 2:
                        nc.tensor.matmul(zz, _pair(xjb, mstat), _pair(sjm, mka),
                                         start=False, stop=True, perf_mode=DR)
                # evict the chunk-pair: relu (+mask already in PSUM).
                # GPSIMD cannot read PSUM, so only ACT/DVE evict (9:7 split).
                on_act = cp % 2 == 0 or cp == 15
                if l < 2:
                    dst = mva[:, msg_off + cp * 1024:msg_off + (cp + 1) * 1024]
                    if on_act:
                        nc.scalar.activation(dst, z[:], AF.Relu)
                    else:
                        nc.vector.tensor_scalar(dst, z[:], 0.0, None, OP.max)
                else:
                    acc = hsum[g][:, cp:cp + 1]
                    if on_act:
                        scr = mva[:, O_SCR:O_SCR + 1024]
                        nc.scalar.activation(scr, z[:], AF.Relu, accum_out=acc)
                    else:
                        scr = mva[:, O_SCR + 1024:O_SCR + 2048]
                        nc.vector.tensor_scalar(scr, z[:], 0.0, None, OP.max,
                                                op1=OP.add, accum_out=acc)

        def xi_xj(g, l):
            """xi' = x@Wi + b, xj' = x@Wj as fp8 stationaries in the arena."""
            xT = xTs[g]
            K = DN0 if l == 0 else DH
            wcol = l * 384 if l < 2 else 768
            ps = xpool.tile([128, 256], F32, tag="xixj")
            nc.tensor.matmul(ps[:, 0:128], xT[:], wbf[0:K, wcol:wcol + 128],
                             start=True, stop=True)
            nc.tensor.matmul(ps[:, 128:256], xT[:], wbf[0:K, wcol + 128:wcol + 256],
                             start=True, stop=True)
            dyn = S_DYN[g]
            nc.vector.tensor_tensor(sta[:, dyn + l * 256:dyn + l * 256 + 128],
                                    ps[:, 0:128], brep[:, l * 128:(l + 1) * 128],
                                    op=OP.add)
            nc.vector.tensor_copy(sta[:, dyn + l * 256 + 128:dyn + l * 256 + 256],
                                  ps[:, 128:256])

        def node_update(g, l):
            """x_new^T = relu(Wx^T x^T + Wa^T agg^T + b) on PE via j-block DRs."""
            xT = xTs[g]
            K = DN0 if l == 0 else DH
            wcol = l * 384 + 256
            xn = npool.tile([128, 128], F32, tag="xn")
            nc.tensor.matmul(xn[:], wbf[0:K, wcol:wcol + 128], xT[:],
                             start=True, stop=False)
            wa = sta[:, S_WAHI[l]:S_WAHI[l] + 256].rearrange(
                "p (two m) -> p two m", two=2)
            wl = sta[:, S_WAHI[l] + 256:S_WAHI[l] + 512].rearrange(
                "p (two m) -> p two m", two=2)
            mo = O_MSG[g][l]
            for b2 in range(64):
                mb = mva[:, mo + b2 * 256:mo + (b2 + 1) * 256].rearrange(
                    "p (two m) -> p two m", two=2)
                nc.tensor.matmul(xn[:], wa, mb, start=False, stop=False,
                                 perf_mode=DR)
                nc.tensor.matmul(xn[:], wl, mb, start=False, stop=(b2 == 63),
                                 perf_mode=DR)
            xnT = small.tile([128, 128], BF16, tag=f"xnT{g}_{l}")
            nc.scalar.activation(xnT[:], xn[:], AF.Relu, bias=bcol[:, l:l + 1])
            if l == 1:
                xbl = small.tile([128, 128], BF16, tag=f"xbl{g}")
                nc.vector.tensor_add(xbl[:], xnT[:], xTs[g][:])
                xTs[g] = xbl
            else:
                xTs[g] = xnT

        def head(g):
            hpre = small.tile([128, 1], F32, tag=f"hp{g}")
            nc.vector.tensor_reduce(hpre[:], hsum[g][:], axis=mybir.AxisListType.X,
                                    op=OP.add)
            h1p = npool.tile([128, 128], F32, tag="xn")
            nc.tensor.matmul(h1p[:, 0:1], whd[:, 0:128], hpre[:],
                             start=True, stop=True)
            h1 = small.tile([128, 1], F32, tag=f"h1{g}")
            nc.scalar.activation(h1[:], h1p[:, 0:1], AF.Relu,
                                 bias=bcol[:, 2:3], scale=1.0 / FREE)
            h2p = npool.tile([128, 128], F32, tag="xn")
            nc.tensor.matmul(h2p[:, 0:1], whd[:, 128:256], h1[:],
                             start=True, stop=True)
            h2 = small.tile([128, 1], F32, tag=f"h2{g}")
            nc.scalar.activation(h2[:], h2p[:, 0:1], AF.Relu, bias=bcol[:, 3:4])
            h3p = npool.tile([128, 128], F32, tag="xn")
            nc.tensor.matmul(h3p[0:1, 0:1], whd[:, 256:257], h2[:],
                             start=True, stop=True)
            oval = small.tile([1, 1], F32, tag=f"ov{g}")
            nc.scalar.activation(oval[:], h3p[0:1, 0:1], AF.Identity, bias=bh3[:])
            nc.sync.dma_start(out_d[g:g + 1, :], oval[:])

        # ---- schedule: layer-interleaved across the two graphs ----
        for l in range(3):
            for g in range(GPC):
                xi_xj(g, l)
                msg_layer(g, l)
            if l < 2:
                for g in range(GPC):
                    node_update(g, l)
        for g in range(GPC):
            head(g)

        stack.close()
    nc.finalize()
    return nc


def _f8(x):
    return np.asarray(x, dtype=np.float32).astype(NPF8)


def prep_core_inputs(inputs, core):
    """Host-side layout/dtype prep for one core's GPC graphs."""
    gs = slice(core * GPC, (core + 1) * GPC)
    A = np.asarray(inputs["edge_index"][gs], np.float32)        # [GPC,N,N]
    x = np.asarray(inputs["x"][gs], np.float32)                 # [GPC,N,DN0]
    ea = np.asarray(inputs["edge_attr"][gs], np.float32)        # [GPC,N,N,DE0]
    w = {k: np.asarray(inputs[k], np.float32) for k in WEIGHT_NAMES}

    f = np.arange(FREE)
    c1f, tf, pf = f // 2048, (f // 128) % 16, f % 128
    jf = 8 * tf + c1f

    # moving constants: seljm | seli; notA rows separately
    mv = np.zeros((128, FREE + CH), NPF8)
    mv[:, 0:FREE] = (np.arange(128)[:, None] == jf[None, :]).astype(NPF8)
    seli = (np.arange(128)[:, None] == (np.arange(CH) % 128)[None, :])
    mv[:, FREE:FREE + CH] = seli.astype(NPF8)
    marow = np.zeros((GPC, FREE), NPF8)
    for g in range(GPC):
        marow[g] = (4.0 * (1.0 - A[g][pf, jf])).astype(NPF8)

    # stationary constants
    st = np.zeros((128, S_DYN[1] - S_WE0), NPF8)
    o = -S_WE0
    We0 = w["w_msg_0"][2 * DN0:]                                # [16,128]
    for c1 in range(8):
        st[16 * c1:16 * (c1 + 1), o + S_WE0 + c1 * 128:o + S_WE0 + (c1 + 1) * 128] \
            = _f8(We0)
    st[:, o + S_WE1:o + S_WE1 + 128] = _f8(w["w_msg_1"][2 * DH:])
    we2h = _f8(0.5 * w["w_msg_2"][2 * DH:])
    st[:, o + S_WE2H2:o + S_WE2H2 + 128] = we2h
    st[:, o + S_WE2H2 + 128:o + S_WE2H2 + 256] = we2h
    for l in range(2):
        Dn = DN0 if l == 0 else DH
        Wa = w[f"w_node_{l}"][Dn:]
        hi = _f8(Wa)
        lo = _f8(Wa - hi.astype(np.float32))
        base = o + S_WAHI[l]
        st[:, base:base + 128] = hi
        st[:, base + 128:base + 256] = hi
        st[:, base + 256:base + 384] = lo
        st[:, base + 384:base + 512] = lo
    for g in range(GPC):
        st[g, o + S_MSTAT + g * 128:o + S_MSTAT + (g + 1) * 128] = \
            np.asarray(-240.0, NPF8)

    # transposed fp8 edge features: [(j8,de), (t16,i)]
    e0t = np.ascontiguousarray(
        ea.reshape(GPC, N, 16, 8, DE0).transpose(0, 3, 4, 2, 1)
    ).reshape(GPC, 128, 2048).astype(NPF8)

    xt = np.ascontiguousarray(x.transpose(0, 2, 1)).astype(NPBF)

    # bf16 x-path weights: per layer [Wi | Wj | Wx], L2 has 0.5-folded Wi/Wj
    wbf = np.zeros((128, 1024), NPBF)
    for l in range(2):
        Dn = DN0 if l == 0 else DH
        wm, wn = w[f"w_msg_{l}"], w[f"w_node_{l}"]
        wbf[0:Dn, l * 384:l * 384 + 128] = wm[0:Dn].astype(NPBF)
        wbf[0:Dn, l * 384 + 128:l * 384 + 256] = wm[Dn:2 * Dn].astype(NPBF)
        wbf[0:Dn, l * 384 + 256:l * 384 + 384] = wn[0:Dn].astype(NPBF)
    wbf[0:DH, 768:896] = (0.5 * w["w_msg_2"][0:DH]).astype(NPBF)
    wbf[0:DH, 896:1024] = (0.5 * w["w_msg_2"][DH:2 * DH]).astype(NPBF)

    brep = np.zeros((128, 384), np.float32)
    for l in range(3):
        brep[:, l * 128:(l + 1) * 128] = w[f"b_msg_{l}"][None, :]
    bcol = np.stack([w["b_node_0"], w["b_node_1"], w["b_h1"], w["b_h2"]],
                    axis=1).astype(np.float32)
    whd = np.zeros((128, 257), np.float32)
    whd[:, 0:128] = w["w_h1"]
    whd[:, 128:256] = w["w_h2"]
    whd[:, 256:257] = w["w_h3"]
    bh3 = w["b_h3"].reshape(1, 1).astype(np.float32)

    return {
        "mvconst": mv, "marow": marow, "stconst": st, "e0t": e0t, "xt": xt,
        "wbf": wbf, "brep": brep, "bcol": bcol, "whd": whd, "bh3": bh3,
    }


def kernel(**inputs):
    inputs = {k: np.asarray(v) for k, v in inputs.items()}
    if "nc" not in _CACHE:
        _CACHE["nc"] = build_nc()
    nc = _CACHE["nc"]

    in_maps = [prep_core_inputs(inputs, c) for c in range(NCORES)]

    from concourse.bass_utils import run_bass_kernel_spmd
    res = run_bass_kernel_spmd(nc, in_maps, list(range(NCORES)))
    out = np.concatenate([np.asarray(res.results[c]["out"]).reshape(-1)
                          for c in range(NCORES)])
    return out.astype(np.float32)
